# revision 1
# baseline (speedup 1.0000x reference)
"""Tricubic B-spline grid interpolation (CubicBSplineGrid3d) on 8 Trainium2 cores.

Strategy (data-parallel over queries, per sharding hint):
  * Host: pad grid (64,64,64,32) -> (67,67,67,32) edge-replicated, then pack the
    4x4 (d,h)-neighborhood redundantly:
        Q[d, h, w, i, j, c] = Gpad[d+i, h+j, w, c]   (f32, ~2.25 GB)
    so each query's full 4x4x4x32 neighborhood is ONE 8KB contiguous run
    Q.flat[base*2048 : base*2048+2048], base=(sd*64+sh)*67+sw.  Exactly the
    information-minimal 8KB/query is fetched, in one 8KB descriptor/partition.
  * Device (per core, 16384 queries = 128 blocks of 128):
      stage 1: compute floor/frac of u*63, cubic basis weights, gather base
               indices; PE-transpose block-layout -> query-on-partition layout.
      main loop: one indirect DMA gathers a [128, 2048] tile (query on
               partition); 16 fused mul-add DVE ops contract (d,h) with
               per-partition scalars; 4 ops contract w; DMA out [128, 32].
  * Host: concat the 8 cores' outputs.
"""
import sys

for _p in ("/opt/trn_rl_repo",):
    if _p not in sys.path:
        sys.path.insert(0, _p)

import numpy as np

N_CORES = 8
B_GLOBAL = 131072
B_LOCAL = B_GLOBAL // N_CORES          # 16384
NBLK = B_LOCAL // 128                  # 128 blocks of 128 queries
GD = GH = GW = 64                      # grid spatial dims
GC = 32                                # channels
QROWS = GD * GH * (GW + 3)             # 64*64*67 = 274432
QROWLEN = 4 * 4 * GC                   # 512 elements = 2KB per (d,h,w) row
GLEN = 4 * QROWLEN                     # 2048 elements = 8KB gathered (4 w-rows)

_nc_cache = None


def _build_nc():
    """Build + compile the per-core Bass program (identical on all cores)."""
    from concourse import bacc, mybir
    from concourse.bass import IndirectOffsetOnAxis
    from concourse.tile import TileContext
    from concourse.masks import make_identity

    f32, i32 = mybir.dt.float32, mybir.dt.int32
    Alu = mybir.AluOpType
    Act = mybir.ActivationFunctionType
    P = 128

    nc = bacc.Bacc("TRN2", target_bir_lowering=False, debug=False,
                   num_devices=N_CORES)
    u_t = nc.dram_tensor("u", [B_LOCAL, 3], f32, kind="ExternalInput")
    q_t = nc.dram_tensor("q", [QROWS, QROWLEN], f32, kind="ExternalInput")
    o_t = nc.dram_tensor("o", [B_LOCAL, GC], f32, kind="ExternalOutput")

    with TileContext(nc) as tc:
        with (
            tc.tile_pool(name="persist", bufs=1) as pp,
            tc.tile_pool(name="stage1", bufs=1) as s1,
            tc.tile_pool(name="psum", bufs=2, space="PSUM") as psum,
            tc.tile_pool(name="g", bufs=6) as gp,
            tc.tile_pool(name="acc", bufs=3) as ap_,
            tc.tile_pool(name="o", bufs=4) as op_,
        ):
            # ---------- stage 1: weights + indices (block layout) ----------
            # U[p, n, a] = u[p*128 + n, a]; per-partition 1536B contiguous.
            U = s1.tile([P, 384], f32)
            nc.sync.dma_start(
                out=U[:, :], in_=u_t[:, :].rearrange("(p n) c -> p (n c)", p=P))
            X = s1.tile([P, 384], f32)
            nc.vector.tensor_scalar(X[:, :], U[:, :], float(GD - 1), None, Alu.mult)
            # floor via round-to-nearest cast + correction
            Si = s1.tile([P, 384], i32)
            nc.vector.tensor_copy(out=Si[:, :], in_=X[:, :])
            Sf = s1.tile([P, 384], f32)
            nc.vector.tensor_copy(out=Sf[:, :], in_=Si[:, :])
            D = s1.tile([P, 384], f32)
            nc.vector.tensor_tensor(out=D[:, :], in0=X[:, :], in1=Sf[:, :],
                                    op=Alu.subtract)
            M = s1.tile([P, 384], f32)
            nc.vector.tensor_scalar(M[:, :], D[:, :], 0.0, None, Alu.is_lt)
            S = s1.tile([P, 384], f32)
            nc.vector.tensor_tensor(out=S[:, :], in0=Sf[:, :], in1=M[:, :],
                                    op=Alu.subtract)
            T = s1.tile([P, 384], f32)
            nc.vector.tensor_tensor(out=T[:, :], in0=X[:, :], in1=S[:, :],
                                    op=Alu.subtract)

            S3 = S[:, :].rearrange("p (n c) -> p n c", c=3)
            # base = (sd*64 + sh)*67 + sw
            Bse = s1.tile([P, 128], f32)
            nc.vector.scalar_tensor_tensor(
                out=Bse[:, :], in0=S3[:, :, 0], scalar=float(GH),
                in1=S3[:, :, 1], op0=Alu.mult, op1=Alu.add)
            nc.vector.scalar_tensor_tensor(
                out=Bse[:, :], in0=Bse[:, :], scalar=float(GW + 3),
                in1=S3[:, :, 2], op0=Alu.mult, op1=Alu.add)

            # cubic basis weights on [128, 384] (all 3 axes at once)
            T2 = s1.tile([P, 384], f32)
            nc.vector.tensor_tensor(out=T2[:, :], in0=T[:, :], in1=T[:, :],
                                    op=Alu.mult)
            T3 = s1.tile([P, 384], f32)
            nc.vector.tensor_tensor(out=T3[:, :], in0=T2[:, :], in1=T[:, :],
                                    op=Alu.mult)
            sixth = 1.0 / 6.0
            W0 = s1.tile([P, 384], f32)
            nc.vector.tensor_scalar(W0[:, :], T3[:, :], -sixth, None, Alu.mult)
            nc.vector.scalar_tensor_tensor(out=W0[:, :], in0=T2[:, :], scalar=0.5,
                                           in1=W0[:, :], op0=Alu.mult, op1=Alu.add)
            nc.vector.scalar_tensor_tensor(out=W0[:, :], in0=T[:, :], scalar=-0.5,
                                           in1=W0[:, :], op0=Alu.mult, op1=Alu.add)
            nc.vector.tensor_scalar(W0[:, :], W0[:, :], sixth, None, Alu.add)
            W1 = s1.tile([P, 384], f32)
            nc.vector.tensor_scalar(W1[:, :], T3[:, :], 0.5, None, Alu.mult)
            nc.vector.scalar_tensor_tensor(out=W1[:, :], in0=T2[:, :], scalar=-1.0,
                                           in1=W1[:, :], op0=Alu.mult, op1=Alu.add)
            nc.vector.tensor_scalar(W1[:, :], W1[:, :], 2.0 / 3.0, None, Alu.add)
            W3 = s1.tile([P, 384], f32)
            nc.vector.tensor_scalar(W3[:, :], T3[:, :], sixth, None, Alu.mult)
            # w2 = 1 - w0 - w1 - w3  (partition of unity)
            W2 = s1.tile([P, 384], f32)
            nc.vector.tensor_tensor(out=W2[:, :], in0=W0[:, :], in1=W1[:, :],
                                    op=Alu.add)
            nc.vector.tensor_tensor(out=W2[:, :], in0=W2[:, :], in1=W3[:, :],
                                    op=Alu.add)
            nc.vector.tensor_scalar(W2[:, :], W2[:, :], -1.0, 1.0,
                                    Alu.mult, Alu.add)

            # ---------- transposes to query-on-partition layout ----------
            ident = pp.tile([P, P], f32)
            make_identity(nc, ident[:, :])

            TD = pp.tile([P, 512], f32)   # wd_i  at cols i*128 + b
            TH = pp.tile([P, 512], f32)   # wh_j  at cols j*128 + b
            TW = pp.tile([P, 512], f32)   # ww_k  at cols k*128 + b
            FB = pp.tile([P, 128], f32)   # base  [query, block]
            Ws = [W0, W1, W2, W3]

            def transpose_into(dst_ap, src_ap):
                pt = psum.tile([P, P], f32, space="PSUM")
                nc.tensor.transpose(out=pt[:, :], in_=src_ap, identity=ident[:, :])
                nc.vector.tensor_copy(out=dst_ap, in_=pt[:, :])

            for a, Tt in ((0, TD), (1, TH), (2, TW)):
                for i in range(4):
                    w3v = Ws[i][:, :].rearrange("p (n c) -> p n c", c=3)
                    transpose_into(Tt[:, i * 128:(i + 1) * 128], w3v[:, :, a])
            transpose_into(FB[:, :], Bse[:, :])

            IdxI = pp.tile([P, 128], i32)
            nc.vector.tensor_copy(out=IdxI[:, :], in_=FB[:, :])

            # wdh_all[q, (i*4+j)*128 + b] = wd_i[q,b] * wh_j[q,b]
            WDH = pp.tile([P, 2048], f32)
            for i in range(4):
                for j in range(4):
                    nc.vector.tensor_tensor(
                        out=WDH[:, (i * 4 + j) * 128:(i * 4 + j + 1) * 128],
                        in0=TD[:, i * 128:(i + 1) * 128],
                        in1=TH[:, j * 128:(j + 1) * 128],
                        op=Alu.mult)

            # ---------- main loop over 128 query blocks ----------
            # G run layout per partition: [w(4), c(32), ij(16)] contiguous.
            # pass A: A4[blk] = sum_k ww_k * G[:, k*512:(k+1)*512]   (w contract)
            # pass B (batched over 4 blocks): o = reduce_ij(A4 * wdh) (d,h)
            WDHv = WDH[:, :].rearrange("p (ij b) -> p b ij", b=128)
            for b in range(NBLK):
                blk = b % 4
                G = gp.tile([P, GLEN], f32)
                nc.gpsimd.indirect_dma_start(
                    out=G[:, :],
                    out_offset=None,
                    in_=q_t[:, :],
                    in_offset=IndirectOffsetOnAxis(ap=IdxI[:, b:b + 1], axis=0),
                )
                if blk == 0:
                    A4 = ap_.tile([P, 4 * QROWLEN], f32)
                Asl = A4[:, blk * QROWLEN:(blk + 1) * QROWLEN]
                nc.vector.tensor_scalar(Asl, G[:, 0:QROWLEN],
                                        TW[:, b:b + 1], None, Alu.mult)
                for k in range(1, 4):
                    nc.vector.scalar_tensor_tensor(
                        out=Asl, in0=G[:, k * QROWLEN:(k + 1) * QROWLEN],
                        scalar=TW[:, k * 128 + b:k * 128 + b + 1],
                        in1=Asl, op0=Alu.mult, op1=Alu.add)
                if blk == 3:
                    b0 = b - 3
                    A4v = A4[:, :].rearrange("p (blk c ij) -> p blk c ij",
                                             blk=4, ij=16)
                    wb = (WDHv[:, b0:b0 + 4, :]
                          .rearrange("p blk (x ij) -> p blk x ij", x=1)
                          .to_broadcast([P, 4, GC, 16]))
                    Pm = ap_.tile([P, 4 * QROWLEN], f32)
                    Pm4 = Pm[:, :].rearrange("p (blk c ij) -> p blk c ij",
                                             blk=4, ij=16)
                    nc.vector.tensor_tensor(out=Pm4[:, :, :, :],
                                            in0=A4v[:, :, :, :],
                                            in1=wb, op=Alu.mult)
                    o4 = op_.tile([P, 4, GC], f32)
                    nc.vector.tensor_reduce(
                        out=o4[:, :, :], in_=Pm4[:, :, :, :],
                        axis=mybir.AxisListType.X, op=Alu.add)
                    nc.sync.dma_start(
                        out=o_t[b0 * 128:(b + 1) * 128, :].rearrange(
                            "(blk q) c -> q blk c", blk=4),
                        in_=o4[:, :, :])
    nc.compile()
    return nc


def _pack_grid(grid: np.ndarray) -> np.ndarray:
    """(64,64,64,32) -> [QROWS, QROWLEN] f32 with
    Q[d,h,w,i,j,c] = Gpad[d+i, h+j, w, c]."""
    gp = np.pad(grid, ((1, 2), (1, 2), (1, 2), (0, 0)), mode="edge")
    win = np.lib.stride_tricks.sliding_window_view(gp, (4, 4), axis=(0, 1))
    # win: [64, 64, 67, 32, 4, 4] = (d, h, w, c, i, j); keep ij innermost so
    # the on-device (d,h) contraction can use tensor_reduce over X.
    q = np.ascontiguousarray(win, dtype=np.float32)
    return q.reshape(QROWS, QROWLEN)


def kernel(u: np.ndarray, grid: np.ndarray) -> np.ndarray:
    global _nc_cache
    from concourse.bass_utils import run_bass_kernel_spmd

    assert u.shape == (B_GLOBAL, 3) and grid.shape == (GD, GH, GW, GC)
    if _nc_cache is None:
        _nc_cache = _build_nc()
    nc = _nc_cache

    q = _pack_grid(np.asarray(grid, dtype=np.float32))
    u = np.ascontiguousarray(u, dtype=np.float32)
    in_maps = [
        {"u": u[c * B_LOCAL:(c + 1) * B_LOCAL], "q": q} for c in range(N_CORES)
    ]
    res = run_bass_kernel_spmd(nc, in_maps, core_ids=list(range(N_CORES)))
    out = np.concatenate([res.results[c]["o"] for c in range(N_CORES)], axis=0)
    return out.astype(np.float32)


if __name__ == "__main__":
    # quick self-run with random inputs
    rng = np.random.default_rng(0)
    grid = rng.standard_normal((GD, GH, GW, GC), dtype=np.float32)
    u = rng.random((B_GLOBAL, 3), dtype=np.float32)
    out = kernel(u, grid)
    print("out", out.shape, out.dtype, float(np.abs(out).mean()))

